# revision 11
# baseline (speedup 1.0000x reference)
"""GCN (3-layer + linear head) on 8 Trainium2 NeuronCores.

Strategy (graph/data parallel, per the sharding hint):
  - Nodes are partitioned across the 8 cores by contiguous range (6250 each,
    padded to 6272 = 49*128). Each core owns the aggregation (scatter targets)
    for its node range; the small weight matrices are replicated.
  - Per layer: each core computes h' = (x_local @ W) * dinv_local, then two
    AllGathers replicate the full scaled feature table (split in half so each
    collective overlaps with gather traffic, and so table row ids fit the
    int16 index format of the gather engine); each core then gathers rows for
    the sources of its incoming edges with batched dma_gather ops across 4
    SWDGE queues.
  - The scatter-add (segment sum by destination) is done on the TensorEngine:
    edges are bucketed by destination tile (128 nodes) with a per-tile chunk
    count derived from the actual edge counts (max across cores, so one SPMD
    program serves all 8); a host-built 0/1 selection matrix per chunk turns
    segment-sum into sel^T @ gathered_rows accumulated in PSUM.
  - Self-loop messages never touch the gather path: the local h' tiles stay
    resident in SBUF and are added when the low-half partial aggregate is
    staged. The residual x @ R is pre-scaled by deg^{+1/2} so the final
    deg^{-1/2} post-scale and relu collapse into a single scalar-engine
    activation with a per-partition scale vector.
  - Layer pipelining: phase A of layer l+1 (x @ W matmuls, table writes) and
    its two AllGathers are emitted inline into layer l's high-half loop, so
    collectives and matmuls hide under the previous layer's gather drain.

The program is compiled per edge structure (chunk plan baked in) and cached
by a hash of edge_index. Host-side work is limited to index preprocessing
(bucketing, padding, selection-matrix layout); all per-feature numeric work
runs on device.
"""

import hashlib
import os
import numpy as np

import concourse.bass as bass
import concourse.tile as tile
from concourse import bacc, mybir
from concourse.bass_utils import run_bass_kernel_spmd
from concourse.masks import make_identity

# ---- problem constants (hardcoded per contract) ----
N = 50000
E = 600000
D = 128
P = 128
NCORES = 8
NPC = N // NCORES            # 6250 nodes per core
NTILE = 49                   # ceil(6250/128)
NLOC = NTILE * P             # 6272 padded local nodes
HLOC = NLOC // 2             # 3136 rows per table half
HTAB = NCORES * HLOC         # 25088 rows per gathered half-table
MAXCH = 8                    # chunks (128 gathered rows) per gather op

f32 = mybir.dt.float32
i16 = mybir.dt.int16
bf16 = mybir.dt.bfloat16
f8 = mybir.dt.float8e4

_CACHE = {}


def _install_ntff_hook():
    """Best-effort NTFF profiling hook (used only when BASS_GCN_TRACE=1)."""
    try:
        import sys, types
        from trn_agent_boot.trn_boot import _ntff_profile_via_ctypes
        hook = _ntff_profile_via_ctypes('/opt/axon/libaxon_pjrt.so')
        if hook is None:
            return
        mod = types.ModuleType("antenv.axon_hooks")
        mod.get_axon_ntff_profile_hook = lambda: hook
        import antenv
        sys.modules['antenv.axon_hooks'] = mod
        antenv.axon_hooks = mod
    except Exception:
        pass


class Plan:
    """Edge-structure-dependent program layout, shared by all cores.

    chunks[h][t]: gathered 128-row chunks for dst tile t, half h.
    starts[h][t]: chunk offset of tile t within half h's chunk sequence.
    tot[h]: total chunks in half h.
    ops[h]: list of (tile_lo, tile_hi, chunk_start, n_chunks) gather batches.
    """

    def __init__(self, counts):
        # counts[h][c][t]: edges per (half, core, tile)
        self.chunks = []
        self.starts = []
        self.tot = []
        self.ops = []
        for h in range(2):
            cmax = counts[h].max(axis=0)           # per-tile max over cores
            ch = np.maximum(1, -(-cmax // P))      # ceil, at least 1
            if ch.max() > MAXCH:
                raise ValueError(f"tile needs {ch.max()} chunks > {MAXCH}")
            st = np.zeros(NTILE + 1, np.int64)
            np.cumsum(ch, out=st[1:])
            self.chunks.append(ch)
            self.starts.append(st)
            self.tot.append(int(st[-1]))
            ops = []
            t0 = 0
            while t0 < NTILE:
                t1 = t0
                nb = 0
                while t1 < NTILE and nb + ch[t1] <= MAXCH:
                    nb += int(ch[t1])
                    t1 += 1
                ops.append((t0, t1, int(st[t0]), nb))
                t0 = t1
            self.ops.append(ops)


def _build_program(plan):
    """Emit the per-core SPMD Bass program (same NEFF on all 8 cores)."""
    nc = bacc.Bacc(None, target_bir_lowering=False, num_swdge_queues=4)

    tot_lo, tot_hi = plan.tot[0], plan.tot[1]

    # ---- I/O ----
    xT_in = nc.dram_tensor("xT", [P, NLOC], f32, kind="ExternalInput")
    dinv_in = nc.dram_tensor("dinv", [P, NTILE], f32, kind="ExternalInput")
    dsq_in = nc.dram_tensor("dsq", [P, NTILE], f32, kind="ExternalInput")
    gidx_lo_in = nc.dram_tensor("gidx_lo", [P, tot_lo * P // 16], i16,
                                kind="ExternalInput")
    gidx_hi_in = nc.dram_tensor("gidx_hi", [P, tot_hi * P // 16], i16,
                                kind="ExternalInput")
    osel_lo_in = nc.dram_tensor("osel_lo", [P, tot_lo * P], f8,
                                kind="ExternalInput")
    osel_hi_in = nc.dram_tensor("osel_hi", [P, tot_hi * P], f8,
                                kind="ExternalInput")
    w_in = {}
    for wn in ("W1", "R1", "W2", "R2", "W3", "Wh"):
        w_in[wn] = nc.dram_tensor(wn, [D, D], f32, kind="ExternalInput")
    bh_in = nc.dram_tensor("bh", [1, D], f32, kind="ExternalInput")
    y_out = nc.dram_tensor("y", [NLOC, D], f32, kind="ExternalOutput")

    # ---- internal DRAM: per-layer collective buffers ----
    srcs, dlo, dhi = [], [], []
    for l in range(3):
        srcs.append(nc.dram_tensor(f"agsrc{l}", [NLOC, D], bf16))
        dlo.append(nc.dram_tensor(f"aglo{l}", [HTAB, D], bf16,
                                  addr_space="Shared"))
        dhi.append(nc.dram_tensor(f"aghi{l}", [HTAB, D], bf16,
                                  addr_space="Shared"))

    layers = [
        ("W1", "R1", True),
        ("W2", "R2", True),
        ("W3", None, False),
    ]

    with tile.TileContext(nc) as tc:
        with tc.tile_pool(name="const", bufs=1) as cp, \
             tc.tile_pool(name="state", bufs=1) as sp, \
             tc.tile_pool(name="glo", bufs=8) as glp, \
             tc.tile_pool(name="ghi", bufs=8) as ghp, \
             tc.tile_pool(name="opool", bufs=3) as op, \
             tc.tile_pool(name="opoolh", bufs=3) as oph, \
             tc.tile_pool(name="work", bufs=6) as wp, \
             tc.tile_pool(name="psA", bufs=2, space="PSUM") as psA, \
             tc.tile_pool(name="psB", bufs=3, space="PSUM") as psB, \
             tc.tile_pool(name="psT", bufs=1, space="PSUM") as psT, \
             tc.tile_pool(name="psR", bufs=2, space="PSUM") as psR:

            # ---- resident state ----
            ident = cp.tile([P, P], f32)
            make_identity(nc, ident[:])
            ones1 = cp.tile([1, P], f32)
            nc.vector.memset(ones1[:], 1.0)
            W = {}
            for wn in ("W1", "R1", "W2", "R2", "W3", "Wh"):
                W[wn] = cp.tile([D, D], f32, name=wn + "_t", tag=wn)
                nc.sync.dma_start(out=W[wn][:], in_=w_in[wn][:])
            bh_t = cp.tile([1, D], f32)
            nc.sync.dma_start(out=bh_t[:], in_=bh_in[:])
            dinv_t = cp.tile([P, NTILE], f32)
            nc.sync.dma_start(out=dinv_t[:], in_=dinv_in[:])
            dsq_t = cp.tile([P, NTILE], f32)
            nc.sync.dma_start(out=dsq_t[:], in_=dsq_in[:])
            gidx_lo = cp.tile([P, tot_lo * P // 16], i16)
            nc.sync.dma_start(out=gidx_lo[:], in_=gidx_lo_in[:])
            gidx_hi = cp.tile([P, tot_hi * P // 16], i16)
            nc.sync.dma_start(out=gidx_hi[:], in_=gidx_hi_in[:])

            xT = sp.tile([P, NLOC], f32)      # resident x^T (feat-major)
            nc.sync.dma_start(out=xT[:, 0:HLOC], in_=xT_in[:, 0:HLOC])
            nc.sync.dma_start(out=xT[:, HLOC:NLOC], in_=xT_in[:, HLOC:NLOC])
            hplus = sp.tile([P, NLOC], f32)   # residual * deg^{1/2}
            aggL = sp.tile([P, NLOC], f32)    # low-half partial aggregates
            hloc = sp.tile([P, NLOC], bf16)   # local h' tiles (self-loop msgs)

            def ts(t):
                return slice(t * P, (t + 1) * P)

            def phase_a_tile(l, t):
                """x @ W (scaled) into hloc + srcs; x @ R * deg^{1/2}."""
                wn, rn, _ = layers[l]
                pA = psA.tile([P, D], f32, space="PSUM", tag="pA")
                nc.tensor.matmul(out=pA[:], lhsT=xT[:, ts(t)], rhs=W[wn][:],
                                 start=True, stop=True)
                nc.vector.tensor_tensor(
                    out=hloc[:, ts(t)], in0=pA[:],
                    in1=dinv_t[:, t:t + 1].to_broadcast([P, D]),
                    op=mybir.AluOpType.mult)
                nc.scalar.dma_start(out=srcs[l][ts(t), :], in_=hloc[:, ts(t)])
                if rn is not None:
                    pR = psR.tile([P, D], f32, space="PSUM", tag="pR")
                    nc.tensor.matmul(out=pR[:], lhsT=xT[:, ts(t)],
                                     rhs=W[rn][:], start=True, stop=True)
                    nc.vector.tensor_tensor(
                        out=hplus[:, ts(t)], in0=pR[:],
                        in1=dsq_t[:, t:t + 1].to_broadcast([P, D]),
                        op=mybir.AluOpType.mult)

            def emit_ag(l, h):
                nc.gpsimd.collective_compute(
                    "AllGather", mybir.AluOpType.bypass,
                    replica_groups=[list(range(NCORES))],
                    ins=[srcs[l][0:HLOC, :] if h == 0
                         else srcs[l][HLOC:NLOC, :]],
                    outs=[(dlo if h == 0 else dhi)[l][:]],
                )

            def emit_half(l, h):
                """Gathers + aggregation for half h of layer l.

                For h==1 also emits the per-tile epilogue, and interleaves
                phase A / AllGathers of layer l+1.
                """
                rn = layers[l][1]
                act = layers[l][2]
                gp_, op_ = (glp, op) if h == 0 else (ghp, oph)
                osel_in = osel_lo_in if h == 0 else osel_hi_in
                gidx = gidx_lo if h == 0 else gidx_hi
                dtab = (dlo if h == 0 else dhi)[l]
                for j, (t0, t1, cs, nb) in enumerate(plan.ops[h]):
                    g = gp_.tile([P, MAXCH, P], bf16, tag=f"g{h}")
                    nc.gpsimd.dma_gather(
                        out_ap=g[:, :nb, :], in_ap=dtab[:],
                        idxs_ap=gidx[:, cs * (P // 16):(cs + nb) * (P // 16)],
                        num_idxs=nb * P, num_idxs_reg=nb * P, elem_size=D,
                        queue_num=j % 4, single_packet=False)
                    o8 = op_.tile([P, MAXCH * P], f8, tag=f"o8{h}")
                    nc.sync.dma_start(out=o8[:, :nb * P],
                                      in_=osel_in[:, cs * P:(cs + nb) * P])
                    o = op_.tile([P, MAXCH * P], bf16, tag=f"o{h}")
                    nc.scalar.copy(out=o[:, :nb * P], in_=o8[:, :nb * P])
                    for t in range(t0, t1):
                        c0 = plan.starts[h][t] - cs
                        nch = plan.chunks[h][t]
                        pB = psB.tile([P, D], f32, space="PSUM", tag="pB")
                        for k in range(nch):
                            nc.tensor.matmul(
                                out=pB[:],
                                lhsT=o[:, (c0 + k) * P:(c0 + k + 1) * P],
                                rhs=g[:, c0 + k, :],
                                start=(k == 0), stop=(k == nch - 1))
                        if h == 0:
                            # stage lo partial + self-loop message (+ scaled
                            # residual, which the final dinv scale undoes)
                            if rn is not None:
                                xs = wp.tile([P, D], f32, tag="xs")
                                nc.vector.tensor_add(out=xs[:], in0=pB[:],
                                                     in1=hplus[:, ts(t)])
                                nc.vector.tensor_add(out=aggL[:, ts(t)],
                                                     in0=xs[:],
                                                     in1=hloc[:, ts(t)])
                            else:
                                nc.vector.tensor_add(out=aggL[:, ts(t)],
                                                     in0=pB[:],
                                                     in1=hloc[:, ts(t)])
                            continue
                        # ---- hi half: finish aggregate + epilogue
                        xn = wp.tile([P, D], f32, tag="xn")
                        nc.vector.tensor_add(out=xn[:], in0=pB[:],
                                             in1=aggL[:, ts(t)])
                        xf = wp.tile([P, D], f32, tag="xf")
                        nc.scalar.activation(
                            out=xf[:], in_=xn[:],
                            func=(mybir.ActivationFunctionType.Relu if act
                                  else mybir.ActivationFunctionType.Copy),
                            scale=dinv_t[:, t:t + 1])
                        pT = psT.tile([P, P], f32, space="PSUM", tag="pT")
                        nc.tensor.transpose(out=pT[:], in_=xf[:],
                                            identity=ident[:])
                        nc.vector.tensor_copy(out=xT[:, ts(t)], in_=pT[:])

                        if l < 2:
                            phase_a_tile(l + 1, t)
                            if t == 24:
                                emit_ag(l + 1, 0)
                            elif t == NTILE - 1:
                                emit_ag(l + 1, 1)
                        else:
                            pH = psA.tile([P, D], f32, space="PSUM", tag="pA")
                            nc.tensor.matmul(out=pH[:], lhsT=xT[:, ts(t)],
                                             rhs=W["Wh"][:], start=True,
                                             stop=False)
                            nc.tensor.matmul(out=pH[:], lhsT=ones1[:],
                                             rhs=bh_t[:], start=False,
                                             stop=True)
                            yt = wp.tile([P, D], f32, tag="yt")
                            nc.vector.tensor_copy(out=yt[:], in_=pH[:])
                            nc.scalar.dma_start(out=y_out[ts(t), :], in_=yt[:])

            # layer 0 phase A + collectives up front
            for t in range(NTILE):
                phase_a_tile(0, t)
                if t == 24:
                    emit_ag(0, 0)
            emit_ag(0, 1)

            for l in range(3):
                emit_half(l, 0)
                emit_half(l, 1)

    nc.compile()
    return nc


def _pack_gidx(slots):
    """slots: int array (multiple of 16) in op order -> [128, cols] int16.

    dma_gather reads index i of an op at partition i%16, column i//16,
    replicated across the 8 q7 cores (i.e. to all 128 partitions).
    """
    total = len(slots)
    cols = total // 16
    arr = np.empty((16, cols), np.int16)
    arr[np.arange(total) % 16, np.arange(total) // 16] = slots
    return np.ascontiguousarray(np.tile(arr, (8, 1)))


def _preprocess(x, edge_index):
    """Bucket edges (no self-loops); build the plan and per-core input maps."""
    x = np.ascontiguousarray(np.asarray(x, dtype=np.float32))
    ei = np.asarray(edge_index)
    row = ei[0].astype(np.int64)
    col = ei[1].astype(np.int64)

    # deg includes the self-loop (reference appends them before the bincount)
    deg = (np.bincount(col, minlength=N) + 1).astype(np.float32)
    dinv = deg ** -0.5
    dsq = deg ** 0.5

    src_core = row // NPC
    src_i = row - src_core * NPC
    src_hi = src_i >= HLOC
    src_tab = src_core * HLOC + np.where(src_hi, src_i - HLOC, src_i)

    dst_core = col // NPC
    dst_loc = col - dst_core * NPC
    tile_id = dst_loc >> 7
    d_in = dst_loc & 127

    # per (half, core, tile) edge counts -> shared chunk plan
    counts = np.zeros((2, NCORES, NTILE), np.int64)
    np.add.at(counts, (src_hi.astype(np.int64), dst_core, tile_id), 1)
    plan = Plan(counts)

    ins = []
    for c in range(NCORES):
        m = {}
        sel = dst_core == c
        s_tab = src_tab[sel]
        s_hi = src_hi[sel]
        t_id = tile_id[sel]
        dd_in = d_in[sel]

        for h, gname, oname in ((0, "gidx_lo", "osel_lo"),
                                (1, "gidx_hi", "osel_hi")):
            tot = plan.tot[h]
            starts = plan.starts[h]
            slots = np.zeros(tot * P, np.int64)        # pad -> row 0
            osel = np.zeros((P, tot * P), np.float32)  # [lane, chunkpos*P+dst]

            hsel = s_hi == bool(h)
            st = s_tab[hsel]
            td = t_id[hsel]
            dd = dd_in[hsel]
            order = np.argsort(td, kind="stable")
            st, td, dd = st[order], td[order], dd[order]
            cnt = np.bincount(td, minlength=NTILE)
            cstart = np.zeros(NTILE + 1, np.int64)
            np.cumsum(cnt, out=cstart[1:])
            pos = np.arange(len(td)) - cstart[td]      # rank within tile
            gpos = starts[td] * P + pos                # global slot
            if len(gpos) and (pos >= plan.chunks[h][td] * P).any():
                raise ValueError("edge count exceeds planned chunks")
            slots[gpos] = st
            lane = gpos & 127
            chunkpos = gpos >> 7
            osel[lane, chunkpos * P + dd] = 1.0

            m[gname] = _pack_gidx(slots)
            m[oname] = np.ascontiguousarray(osel.astype(mybir.dt.np(f8)))

        xl = np.zeros((NLOC, D), np.float32)
        xl[:NPC] = x[c * NPC:(c + 1) * NPC]
        m["xT"] = np.ascontiguousarray(xl.T)

        dv = np.zeros(NLOC, np.float32)
        dv[:NPC] = dinv[c * NPC:(c + 1) * NPC]
        m["dinv"] = np.ascontiguousarray(dv.reshape(NTILE, P).T)
        ds = np.zeros(NLOC, np.float32)
        ds[:NPC] = dsq[c * NPC:(c + 1) * NPC]
        m["dsq"] = np.ascontiguousarray(ds.reshape(NTILE, P).T)
        ins.append(m)
    return plan, ins


LAST_EXEC_NS = None


def kernel(x, edge_index, W1, R1, W2, R2, W3, Wh, bh):
    global LAST_EXEC_NS
    trace = os.environ.get("BASS_GCN_TRACE", "0") == "1"
    if trace:
        _install_ntff_hook()

    plan, ins = _preprocess(x, edge_index)

    key = hashlib.sha1(np.ascontiguousarray(edge_index).tobytes()).hexdigest()
    if _CACHE.get("key") != key:
        _CACHE["nc"] = _build_program(plan)
        _CACHE["key"] = key
    nc = _CACHE["nc"]

    wmap = {"W1": W1, "R1": R1, "W2": W2, "R2": R2, "W3": W3, "Wh": Wh}
    for m in ins:
        for k, v in wmap.items():
            m[k] = np.ascontiguousarray(np.asarray(v, dtype=np.float32))
        m["bh"] = np.ascontiguousarray(
            np.asarray(bh, dtype=np.float32).reshape(1, D))

    res = run_bass_kernel_spmd(
        nc, ins, core_ids=list(range(NCORES)), trace=trace)
    LAST_EXEC_NS = res.exec_time_ns

    out = np.empty((N, D), np.float32)
    for c in range(NCORES):
        out[c * NPC:(c + 1) * NPC] = res.results[c]["y"][:NPC]
    return out
